# revision 5
# baseline (speedup 1.0000x reference)
"""MultiHeadAttention Trainium2 kernel (v2).

B=4, T=2048, D=512, H=8 heads (head dim 64). 8 NeuronCores.
Sharding: core i handles batch b = i//2, query rows half = i%2 (1024
rows). Each core computes its full attention + output projection slice;
outputs are disjoint so the host just concatenates (no collectives).

Host prep (not counted in HW exec time): q/k/v transposed to [D, t]
fp16; k/v compacted to the unmasked key positions per batch (masked
softmax weights are exactly 0 in the reference), padded to a multiple
of 128 with a 0/1 valid column carried next to v so padded keys drop
out of the softmax denominator.

v2 device-side structure (all matmul operands fp16 unless noted, fp32
PSUM accumulate):
  - ~14 warmup matmuls on a zero tile run while the first DMAs land, so
    the PE HAM clock-gate (1.2 -> 2.4 GHz after ~3.4us busy) is already
    released when real work starts.
  - scores per (head pair, key tile): TWO row-tiled K=64 matmuls at
    tile_position (0,0)/(64,0) — they execute concurrently (measured
    1.6x), writing one [128, 1024] PSUM tile (h0 tq-half | h1 tq-half).
  - exp: softmax scale is folded into the qh projection copy as
    PRESCALE/sqrt(D) (see exp_op.py). Most key tiles exp on the Scalar
    engine via activation(Exp, scale=ACT_SCALE); tiles in DVE_NS exp on
    the Vector engine via the custom EXP4_ANT op — splitting the 9.4M
    exps across both engines instead of serializing on ACT.
  - AV: per head one K=128, M=65 matmul (64 v dims + a valid column
    that accumulates the softmax denominator, excluding padded keys),
    accumulated over key tiles into o_ps[65, 1024] (h0|h1 halves).
  - normalize: 1/sums via reciprocal_approx_fast at partition 0,
    replicated across partitions by a K=1 fp32r ones-row outer product
    on the PE; one fused PSUMxPSUM multiply writes the normalized o^T.
  - out projection per tq half; emission is software-pipelined: the
    next head pair's kh/qh projection matmuls are sprinkled between
    attention groups so the PE never idles while ACT runs exp, and
    AV(n-1) is emitted after scores(n) so the PE doesn't wait on exp.
  - output DMA'd as fp16 (host upcasts; adds ~2e-4 relative error).
"""

import numpy as np
from functools import lru_cache

import concourse.bacc as bacc
import concourse.mybir as mybir
import concourse.tile as tile
from concourse.bass_utils import run_bass_kernel_spmd

from exp_op import exp4, PRESCALE, ACT_SCALE

P = 128
D = 512
NH = 8
C = 64
TQ = 1024  # query rows per core
B, T = 4, 2048
N_CORES = 8
F32 = mybir.dt.float32
F32R = mybir.dt.float32r
F16 = mybir.dt.float16
EXP = mybir.ActivationFunctionType.Exp
SCALE2 = PRESCALE * float(D) ** -0.5
DVE_NS = frozenset({3, 7})  # key tiles whose exp runs on the Vector engine
N_WARM = 14


def _chunks(total, step=D):
    out = []
    o = 0
    while o < total:
        out.append((o, min(step, total - o)))
        o += step
    return out


@lru_cache(maxsize=8)
def _build(KP: int, use_bias: bool = False):
    NK = KP // P
    kch = _chunks(KP)
    nc = bacc.Bacc(None, target_bir_lowering=False, debug=False)

    qt_d = nc.dram_tensor("qt", [D, TQ], F16, kind="ExternalInput")
    kt_d = nc.dram_tensor("kt", [D, KP], F16, kind="ExternalInput")
    vt_d = nc.dram_tensor("vt", [D, KP], F16, kind="ExternalInput")
    wq_d = nc.dram_tensor("wqt", [D, D], F16, kind="ExternalInput")
    wk_d = nc.dram_tensor("wkt", [D, D], F16, kind="ExternalInput")
    wv_d = nc.dram_tensor("wvt", [D, D], F16, kind="ExternalInput")
    wo_d = nc.dram_tensor("wot", [D, D], F16, kind="ExternalInput")
    bias_d = nc.dram_tensor("biases", [1, 4 * D + TQ], F16, kind="ExternalInput")
    val_d = nc.dram_tensor("valid", [KP, NH, 1], F16, kind="ExternalInput")
    bcol_d = nc.dram_tensor("biascol", [P, 8], F32, kind="ExternalInput")
    out_d = nc.dram_tensor("out", [TQ, D], F16, kind="ExternalOutput")

    with tile.TileContext(nc) as tc:
        with (
            tc.tile_pool(name="cst", bufs=1) as cst,
            tc.tile_pool(name="wp", bufs=1) as wp,
            tc.tile_pool(name="xt", bufs=1) as xtp,
            tc.tile_pool(name="pj", bufs=1) as pjp,
            tc.tile_pool(name="vp", bufs=1) as vpp,
            tc.tile_pool(name="at", bufs=4) as atp,
            tc.tile_pool(name="nm", bufs=6) as nmp,
            tc.tile_pool(name="ot", bufs=2) as otp,
            tc.tile_pool(name="rr", bufs=2, space="PSUM") as prr,
            tc.tile_pool(name="pss", bufs=2, space="PSUM") as pss_p,
            tc.tile_pool(name="po", bufs=1, space="PSUM") as po_p,
        ):
            # ---- constants + PE warmup (covers the initial DMA wait) ----
            warm = cst.tile([P, D], F16, tag="warm", name="warm")
            nc.vector.memset(warm, 0.0)
            onescol = cst.tile([1, C], F16, tag="onescol", name="onescol")
            nc.vector.memset(onescol, 1.0)
            bias_sb = cst.tile([1, 4 * D + TQ], F16, tag="bias", name="bias_sb")
            nc.sync.dma_start(out=bias_sb, in_=bias_d[:])
            ones = bias_sb[0:1, 4 * D:4 * D + TQ]
            bcol = cst.tile([P, 8], F32, tag="bcol", name="bcol")
            if use_bias:
                nc.sync.dma_start(out=bcol, in_=bcol_d[:])
            for i in range(N_WARM):
                ps = prr.tile([P, D], F32, tag="rr", name="warm_ps")
                nc.tensor.matmul(ps, warm[:, 0:P], warm, start=True, stop=True)

            # ---- input DMAs, just-in-time order ----
            wk = []
            for kk in range(4):
                t = wp.tile([P, D], F16, tag=f"wk{kk}", name=f"wk{kk}")
                nc.sync.dma_start(out=t, in_=wk_d[kk * P:(kk + 1) * P, :])
                wk.append(t)
            ktc = [[None] * len(kch) for _ in range(4)]
            for ci, (c0, cw) in enumerate(kch):
                for kk in range(4):
                    t = xtp.tile([P, cw], F16, tag=f"kt{kk}_{ci}",
                                 name=f"kt{kk}_{ci}")
                    nc.sync.dma_start(
                        out=t, in_=kt_d[kk * P:(kk + 1) * P, c0:c0 + cw])
                    ktc[kk][ci] = t
            wq = []
            for kk in range(4):
                t = wp.tile([P, D], F16, tag=f"wq{kk}", name=f"wq{kk}")
                nc.sync.dma_start(out=t, in_=wq_d[kk * P:(kk + 1) * P, :])
                wq.append(t)
            qt = []
            for kk in range(4):
                t = xtp.tile([P, TQ], F16, tag=f"qt{kk}", name=f"qt{kk}")
                nc.sync.dma_start(out=t, in_=qt_d[kk * P:(kk + 1) * P, :])
                qt.append(t)
            wv = []
            for kk in range(4):
                t = wp.tile([P, D], F16, tag=f"wv{kk}", name=f"wv{kk}")
                nc.sync.dma_start(out=t, in_=wv_d[kk * P:(kk + 1) * P, :])
                wv.append(t)
            vtc = [[None] * len(kch) for _ in range(4)]
            for ci, (c0, cw) in enumerate(kch):
                for kk in range(4):
                    t = xtp.tile([P, cw], F16, tag=f"vt{kk}_{ci}",
                                 name=f"vt{kk}_{ci}")
                    nc.sync.dma_start(
                        out=t, in_=vt_d[kk * P:(kk + 1) * P, c0:c0 + cw])
                    vtc[kk][ci] = t
            wo = []
            for j in range(4):
                t = wp.tile([P, D], F16, tag=f"wo{j}", name=f"wo{j}")
                nc.sync.dma_start(out=t, in_=wo_d[j * P:(j + 1) * P, :])
                wo.append(t)
            valrep = []
            for n in range(NK):
                t = vpp.tile([P, NH, 1], F16, tag=f"valrep{n}",
                             name=f"valrep{n}")
                nc.sync.dma_start(out=t, in_=val_d[n * P:(n + 1) * P, :, :])
                valrep.append(t)

            # ---- projection emitters (called inline / as fillers) ----
            khT = [pjp.tile([P, KP], F16, tag=f"khT{m}", name=f"khT{m}")
                   for m in range(4)]
            qhT = [pjp.tile([P, TQ], F16, tag=f"qhT{m}", name=f"qhT{m}")
                   for m in range(4)]
            vh = [vpp.tile([P, NH, C + 1], F16, tag=f"vh{n}", name=f"vh{n}")
                  for n in range(NK)]

            def kh_group(m, ci):
                c0, cw = kch[ci]
                ps = prr.tile([P, D], F32, tag="rr", name="kh_ps")
                for kk in range(4):
                    nc.tensor.matmul(ps[:, :cw], wk[kk][:, m * P:(m + 1) * P],
                                     ktc[kk][ci], start=(kk == 0),
                                     stop=(kk == 3))
                if use_bias:
                    nc.vector.tensor_scalar_add(
                        khT[m][:, c0:c0 + cw], ps[:, :cw], bcol[:, 4 + m:5 + m])
                else:
                    nc.vector.tensor_copy(khT[m][:, c0:c0 + cw], ps[:, :cw])

            def qh_group(m, t2):
                ps = prr.tile([P, D], F32, tag="rr", name="qh_ps")
                for kk in range(4):
                    nc.tensor.matmul(ps, wq[kk][:, m * P:(m + 1) * P],
                                     qt[kk][:, t2 * D:(t2 + 1) * D],
                                     start=(kk == 0), stop=(kk == 3))
                if use_bias:
                    nc.vector.tensor_scalar(
                        qhT[m][:, t2 * D:(t2 + 1) * D], ps,
                        bcol[:, m:m + 1], SCALE2,
                        op0=mybir.AluOpType.add, op1=mybir.AluOpType.mult)
                else:
                    nc.vector.tensor_scalar_mul(
                        qhT[m][:, t2 * D:(t2 + 1) * D], ps, SCALE2)

            def vh_group(n):
                ci, off = divmod(n * P, D)
                ps = prr.tile([P, D], F32, tag="rr", name="vh_ps")
                for kk in range(4):
                    nc.tensor.matmul(ps, vtc[kk][ci][:, off:off + P], wv[kk],
                                     start=(kk == 0),
                                     stop=(kk == 3 and not use_bias))
                if use_bias:
                    nc.tensor.matmul(ps, ones[:, 0:P],
                                     bias_sb[0:1, 2 * D:3 * D],
                                     start=False, stop=True)
                nc.vector.tensor_copy(
                    vh[n][:, :, 0:C], ps.rearrange("p (h c) -> p h c", h=NH))
                nc.vector.tensor_copy(vh[n][:, :, C:C + 1], valrep[n])

            # filler queue: proj groups sprinkled between attention groups
            fillers = []
            for m in range(1, 4):
                for ci in range(len(kch)):
                    fillers.append((kh_group, m, ci))
                for t2 in range(2):
                    fillers.append((qh_group, m, t2))

            def run_fillers(k):
                for _ in range(min(k, len(fillers))):
                    fn, *args = fillers.pop(0)
                    fn(*args)

            # ---- head-pair 0 prerequisites ----
            for ci in range(len(kch)):
                kh_group(0, ci)
            for t2 in range(2):
                qh_group(0, t2)
            vh_group(0)
            vh_group(1)

            # ---- attention ----
            onTp = [[nmp.tile([P, D], F16, tag=f"onTp{j}_{t}",
                              name=f"onTp{j}_{t}", bufs=1)
                     for t in range(2)] for j in range(4)]

            def out_proj(t2):
                for tq4 in range(4):
                    tqc = t2 * 4 + tq4
                    ps = prr.tile([P, D], F32, tag="rr", name="out_ps")
                    for j in range(4):
                        nc.tensor.matmul(
                            ps, onTp[j][t2][:, tq4 * P:(tq4 + 1) * P], wo[j],
                            start=(j == 0), stop=(j == 3 and not use_bias))
                    if use_bias:
                        nc.tensor.matmul(ps, ones[:, 0:P],
                                         bias_sb[0:1, 3 * D:4 * D],
                                         start=False, stop=True)
                    osb = otp.tile([P, D], F16, tag="outsb", name="outsb")
                    nc.vector.tensor_copy(osb, ps)
                    nc.sync.dma_start(out=out_d[tqc * P:(tqc + 1) * P, :],
                                      in_=osb)

            pending_outproj = [None]

            for t2 in range(2):
                tsl = slice(t2 * D, (t2 + 1) * D)
                for hp in range(4):
                    o_ps = po_p.tile([C + 1, 2 * D], F32, tag="po",
                                     name="o_ps", bufs=1)
                    prev_av = [None]

                    def emit_av(n, a):
                        for h in range(2):
                            nc.tensor.matmul(
                                o_ps[:, h * D:(h + 1) * D],
                                vh[n][:, 2 * hp + h, :],
                                a[:, h * D:(h + 1) * D],
                                start=(n == 0), stop=(n == NK - 1))

                    for n in range(NK):
                        if t2 == 0 and hp == 0 and n + 2 < NK:
                            vh_group(n + 2)
                        s = pss_p.tile([P, 2 * D], F32, tag="pss", name="s_ps")
                        kb = n * P
                        nc.tensor.matmul(
                            s[:, 0:D], khT[hp][0:C, kb:kb + P],
                            qhT[hp][0:C, tsl], start=True, stop=True)
                        nc.tensor.matmul(
                            s[:, D:2 * D], khT[hp][C:P, kb:kb + P],
                            qhT[hp][C:P, tsl], start=True, stop=True)
                        a = atp.tile([P, 2 * D], F16, tag="aT", name="aT")
                        if n in DVE_NS:
                            exp4(nc, a, s)
                        else:
                            nc.scalar.activation(a, s, EXP, scale=ACT_SCALE)
                        if prev_av[0] is not None:
                            emit_av(*prev_av[0])
                        prev_av[0] = (n, a)
                        if t2 == 0:
                            run_fillers(1 if hp < 3 else 2)
                        elif pending_outproj[0] is not None and n == 1:
                            out_proj(pending_outproj[0])
                            pending_outproj[0] = None
                    emit_av(*prev_av[0])

                    # ---- normalize: o / sums -> onTp (fp16) ----
                    # DVE computes 1/sums; the o PSUM->SBUF staging copy and
                    # the fp16 recip cast ride the Scalar engine's slack.
                    rv = nmp.tile([1, 2 * D], F32, tag="rv", name="rv", bufs=2)
                    nc.vector.tensor_copy(rv, o_ps[C:C + 1, :])
                    osb = nmp.tile([C, 2 * D], F32, tag="osb", name="osb",
                                   bufs=2)
                    nc.scalar.copy(osb, o_ps[0:C, :])
                    rcp = nmp.tile([1, 2 * D], F32, tag="rcp", name="rcp",
                                   bufs=2)
                    nc.vector.reciprocal_approx_fast(out=rcp, in_=rv)
                    rrow = nmp.tile([1, 2 * D], F16, tag="rrow", name="rrow",
                                    bufs=2)
                    nc.scalar.copy(rrow, rcp)
                    for h in range(2):
                        rrep = prr.tile([C, D], F32, tag="rr", name="rrep_ps")
                        nc.tensor.matmul(
                            rrep, onescol, rrow[0:1, h * D:(h + 1) * D],
                            start=True, stop=True)
                        nc.vector.tensor_mul(
                            onTp[hp][t2][h * C:(h + 1) * C, :],
                            osb[:, h * D:(h + 1) * D], rrep)
                run_fillers(len(fillers))
                pending_outproj[0] = t2
            out_proj(1)

    nc.compile()
    return nc


def _prep(q, k, v, mask, Wq, bq, Wk, bk, Wv, bv, Wo, bo):
    q = np.asarray(q, np.float32)
    k = np.asarray(k, np.float32)
    v = np.asarray(v, np.float32)
    mask = np.asarray(mask)
    wqt = np.ascontiguousarray(np.asarray(Wq, np.float32).T.astype(np.float16))
    wkt = np.ascontiguousarray(np.asarray(Wk, np.float32).T.astype(np.float16))
    wvt = np.ascontiguousarray(np.asarray(Wv, np.float32).T.astype(np.float16))
    wot = np.ascontiguousarray(np.asarray(Wo, np.float32).T.astype(np.float16))
    biascol = np.concatenate([
        np.asarray(bq, np.float32).reshape(4, P).T,
        np.asarray(bk, np.float32).reshape(4, P).T], axis=1)
    biascol = np.ascontiguousarray(biascol, dtype=np.float32)
    biases = np.concatenate(
        [np.asarray(x, np.float32) for x in (bq, bk, bv, bo)]
        + [np.ones(TQ, np.float32)]).reshape(1, 4 * D + TQ).astype(np.float16)

    sels = [np.flatnonzero(mask[b]) for b in range(B)]
    kmax = max(1, max(len(s) for s in sels))
    KP = ((kmax + P - 1) // P) * P

    in_maps = []
    for core in range(N_CORES):
        b, half = divmod(core, 2)
        sel = sels[b]
        ns = len(sel)
        kt = np.zeros((D, KP), np.float16)
        kt[:, :ns] = k[b, sel, :].T
        vt = np.zeros((D, KP), np.float16)
        vt[:, :ns] = v[b, sel, :].T
        valid = np.zeros((KP, NH, 1), np.float16)
        valid[:ns] = 1.0
        qt = np.ascontiguousarray(
            q[b, half * TQ:(half + 1) * TQ, :].T.astype(np.float16))
        in_maps.append(dict(
            qt=qt, kt=kt, vt=vt, wqt=wqt, wkt=wkt, wvt=wvt, wot=wot,
            biases=biases, valid=valid, biascol=biascol))
    return KP, in_maps


def kernel(q, k, v, mask, Wq, bq, Wk, bk, Wv, bv, Wo, bo, _bench=[None]):
    KP, in_maps = _prep(q, k, v, mask, Wq, bq, Wk, bk, Wv, bv, Wo, bo)
    use_bias = any(bool(np.any(np.asarray(x))) for x in (bq, bk, bv, bo))
    nc = _build(KP, use_bias)
    res = run_bass_kernel_spmd(nc, in_maps, list(range(N_CORES)))
    _bench[0] = res
    out = np.empty((B, T, D), np.float32)
    for core in range(N_CORES):
        b, half = divmod(core, 2)
        out[b, half * TQ:(half + 1) * TQ, :] = \
            res.results[core]["out"].astype(np.float32)
    return out
